# revision 3
# baseline (speedup 1.0000x reference)
"""DiceLoss kernel for 8 Trainium2 NeuronCores.

Full inputs -> shard x-axis across 8 cores -> per-core Bass kernel computes
per-class partial sums (intersect, sum-of-squares) -> host combines partials
and the label histogram into the final dice loss scalar.

Device traffic is halved by casting activations to bf16 on the host; all
accumulation happens in fp32 on device (accum_out), so the only precision
loss is the input quantization, which averages out over ~450K terms/class.
"""
import numpy as np
import ml_dtypes
import concourse.bacc as bacc
import concourse.mybir as mybir
import concourse.tile as tile
from concourse.bass_utils import run_bass_kernel_spmd

N_CORES = 8
B, C, X, Y, Z = 2, 33, 96, 96, 96
XS = X // N_CORES            # x-planes per core
VOX = XS * Y * Z             # voxels per (b, c) per core = 110592
P = 128
F = VOX // P                 # 864
FB = B * F                   # both batch entries concatenated: 1728
SMOOTH = 1e-5

_cached = {}


def _build():
    nc = bacc.Bacc("TRN2", target_bir_lowering=False, debug=False,
                   num_devices=N_CORES)
    dt = mybir.dt.bfloat16
    x_in = nc.dram_tensor("x", [C, P, FB], dt, kind="ExternalInput")
    lab_in = nc.dram_tensor("lab", [P, FB], dt, kind="ExternalInput")
    stats = nc.dram_tensor("stats", [2, P, C], mybir.dt.float32,
                           kind="ExternalOutput")
    with tile.TileContext(nc) as tc:
        with (
            tc.tile_pool(name="xp", bufs=4) as xp,
            tc.tile_pool(name="labp", bufs=1) as labp,
            tc.tile_pool(name="scr", bufs=3) as scrp,
            tc.tile_pool(name="scr2", bufs=3) as scr2p,
            tc.tile_pool(name="stat", bufs=1) as statp,
        ):
            lab_t = labp.tile([P, FB], dt)
            nc.sync.dma_start(lab_t[:], lab_in[:, :])
            int_pp = statp.tile([P, C], mybir.dt.float32, tag="int")
            sq_pp = statp.tile([P, C], mybir.dt.float32, tag="sq")
            for c in range(C):
                xt = xp.tile([P, FB], dt)
                nc.sync.dma_start(xt[:], x_in[c, :, :])
                scr = scrp.tile([P, FB], dt)
                nc.vector.scalar_tensor_tensor(
                    out=scr[:], in0=lab_t[:], scalar=float(c), in1=xt[:],
                    op0=mybir.AluOpType.is_equal, op1=mybir.AluOpType.mult,
                    accum_out=int_pp[:, c:c + 1])
                scr2 = scr2p.tile([P, FB], dt)
                if c % 3 == 2:
                    # square+reduce on DVE to offload the scalar engine
                    # (tensor_tensor_reduce faults on HW; scalar_tensor_tensor
                    # with bypass/mult computes the same thing)
                    nc.vector.scalar_tensor_tensor(
                        out=scr2[:], in0=xt[:], scalar=0.0, in1=xt[:],
                        op0=mybir.AluOpType.bypass, op1=mybir.AluOpType.mult,
                        accum_out=sq_pp[:, c:c + 1])
                else:
                    nc.scalar.activation(
                        out=scr2[:], in_=xt[:],
                        func=mybir.ActivationFunctionType.Square,
                        accum_out=sq_pp[:, c:c + 1])
            nc.sync.dma_start(stats[0, :, :], int_pp[:])
            nc.sync.dma_start(stats[1, :, :], sq_pp[:])
    nc.compile()
    return nc


def _get_nc():
    if "nc" not in _cached:
        _cached["nc"] = _build()
    return _cached["nc"]


def kernel(outputs, label):
    nc = _get_nc()
    outputs = np.asarray(outputs)
    lab_np = np.asarray(label)
    bf16 = ml_dtypes.bfloat16
    in_maps = []
    for k in range(N_CORES):
        # [B, C, XS, Y, Z] -> [C, B, P, F] -> [C, P, B, F] so each class is
        # one [128, 1728] tile with both batch entries along the free dim.
        xs = outputs[:, :, k * XS:(k + 1) * XS].reshape(B, C, P, F)
        xs = np.ascontiguousarray(xs.transpose(1, 2, 0, 3)).reshape(C, P, FB)
        ls = lab_np[:, k * XS:(k + 1) * XS].reshape(B, P, F)
        ls = np.ascontiguousarray(ls.transpose(1, 0, 2)).reshape(P, FB)
        in_maps.append({"x": xs.astype(bf16),
                        "lab": ls.astype(bf16)})
    res = run_bass_kernel_spmd(nc, in_maps, core_ids=list(range(N_CORES)))
    _cached["last_results"] = res
    intersect = np.zeros(C, np.float64)
    sumsq = np.zeros(C, np.float64)
    for r in res.results:
        st = r["stats"].astype(np.float64)          # [2, P, C]
        intersect += st[0].sum(axis=0)
        sumsq += st[1].sum(axis=0)
    labels_sum = np.bincount(
        lab_np.reshape(-1).astype(np.int64), minlength=C).astype(np.float64)
    dice = (2.0 * intersect + SMOOTH) / (sumsq + labels_sum + SMOOTH)
    return np.float32(np.mean(1.0 - dice))


# revision 5
# speedup vs baseline: 1.2553x; 1.2553x over previous
"""DiceLoss kernel for 8 Trainium2 NeuronCores.

Full inputs -> shard x-axis across 8 cores -> per-core Bass kernel computes
per-class partial sums (intersect, sum-of-squares) -> host combines partials
and the label histogram into the final dice loss scalar.

Device traffic is halved by casting activations to bf16 on the host; all
accumulation happens in fp32 on device (accum_out), so the only precision
loss is the input quantization, which averages out over ~450K terms/class.
"""
import numpy as np
import ml_dtypes
import concourse.bacc as bacc
import concourse.mybir as mybir
import concourse.tile as tile
from concourse.bass_utils import run_bass_kernel_spmd

N_CORES = 8
B, C, X, Y, Z = 2, 33, 96, 96, 96
XS = X // N_CORES            # x-planes per core
VOX = XS * Y * Z             # voxels per (b, c) per core = 110592
P = 128
F = VOX // P                 # 864
FB = B * F                   # both batch entries concatenated: 1728
SMOOTH = 1e-5

_cached = {}


def _build():
    nc = bacc.Bacc("TRN2", target_bir_lowering=False, debug=False,
                   num_devices=N_CORES)
    dt = mybir.dt.bfloat16
    x_in = nc.dram_tensor("x", [C, P, FB], dt, kind="ExternalInput")
    lab_in = nc.dram_tensor("lab", [P, FB], dt, kind="ExternalInput")
    stats = nc.dram_tensor("stats", [2, P, C], mybir.dt.float32,
                           kind="ExternalOutput")
    with tile.TileContext(nc) as tc:
        with (
            tc.tile_pool(name="xp", bufs=4) as xp,
            tc.tile_pool(name="labp", bufs=1) as labp,
            tc.tile_pool(name="scr", bufs=3) as scrp,
            tc.tile_pool(name="scr2", bufs=3) as scr2p,
            tc.tile_pool(name="stat", bufs=1) as statp,
        ):
            lab_t = labp.tile([P, FB], dt)
            nc.sync.dma_start(lab_t[:], lab_in[:, :])
            int_pp = statp.tile([P, C], mybir.dt.float32, tag="int")
            sq_pp = statp.tile([P, C], mybir.dt.float32, tag="sq")
            # Engine balance: DVE runs all 33 intersect ops (STT is 1x-mode,
            # ~1.86us each) plus N_SQ_DVE squares via TT(2x)+TS(4x) (~1.47us);
            # ACT takes the remaining squares (~1.92us each).
            N_SQ_DVE = 1
            for c in range(C):
                xt = xp.tile([P, FB], dt)
                nc.sync.dma_start(xt[:], x_in[c, :, :])
                scr = scrp.tile([P, FB], dt)
                nc.vector.scalar_tensor_tensor(
                    out=scr[:], in0=lab_t[:], scalar=float(c), in1=xt[:],
                    op0=mybir.AluOpType.is_equal, op1=mybir.AluOpType.mult,
                    accum_out=int_pp[:, c:c + 1])
                scr2 = scr2p.tile([P, FB], dt)
                if c < N_SQ_DVE:
                    sq = scrp.tile([P, FB], dt, tag="sqprod")
                    nc.vector.tensor_tensor(sq[:], xt[:], xt[:],
                                            mybir.AluOpType.mult)
                    nc.vector.tensor_scalar(
                        scr2[:], sq[:], 1.0, None, mybir.AluOpType.mult,
                        mybir.AluOpType.add, accum_out=sq_pp[:, c:c + 1])
                else:
                    nc.scalar.activation(
                        out=scr2[:], in_=xt[:],
                        func=mybir.ActivationFunctionType.Square,
                        accum_out=sq_pp[:, c:c + 1])
            nc.sync.dma_start(stats[0, :, :], int_pp[:])
            nc.sync.dma_start(stats[1, :, :], sq_pp[:])
    nc.compile()
    return nc


def _get_nc():
    if "nc" not in _cached:
        _cached["nc"] = _build()
    return _cached["nc"]


def kernel(outputs, label):
    nc = _get_nc()
    outputs = np.asarray(outputs)
    lab_np = np.asarray(label)
    bf16 = ml_dtypes.bfloat16
    in_maps = []
    for k in range(N_CORES):
        # [B, C, XS, Y, Z] -> [C, B, P, F] -> [C, P, B, F] so each class is
        # one [128, 1728] tile with both batch entries along the free dim.
        xs = outputs[:, :, k * XS:(k + 1) * XS].reshape(B, C, P, F)
        xs = np.ascontiguousarray(xs.transpose(1, 2, 0, 3)).reshape(C, P, FB)
        ls = lab_np[:, k * XS:(k + 1) * XS].reshape(B, P, F)
        ls = np.ascontiguousarray(ls.transpose(1, 0, 2)).reshape(P, FB)
        in_maps.append({"x": xs.astype(bf16),
                        "lab": ls.astype(bf16)})
    res = run_bass_kernel_spmd(nc, in_maps, core_ids=list(range(N_CORES)))
    _cached["last_results"] = res
    intersect = np.zeros(C, np.float64)
    sumsq = np.zeros(C, np.float64)
    for r in res.results:
        st = r["stats"].astype(np.float64)          # [2, P, C]
        intersect += st[0].sum(axis=0)
        sumsq += st[1].sum(axis=0)
    labels_sum = np.bincount(
        lab_np.reshape(-1).astype(np.int64), minlength=C).astype(np.float64)
    dice = (2.0 * intersect + SMOOTH) / (sumsq + labels_sum + SMOOTH)
    return np.float32(np.mean(1.0 - dice))
